# revision 66
# baseline (speedup 1.0000x reference)
"""Trainium2 Bass kernel: single-head attention module (dense transformer).

Computes, for x [4, 4096, 256] (f32) and per-projection weights/biases:
    q = x @ Wq + bq;  k = x @ Wk + bk;  v = x @ Wv + bv
    out = softmax((q k^T) / sqrt(256)) @ v @ Wo + bo

Sharding over 8 NeuronCores: core c handles batch c//2, query half c%2.
The host rotates each core's batch so its queries are always rows 0..2047
(softmax is key-order invariant), keeping the device program identical
across cores.

Math rewrite (host-side, weights only):
  scores = (x Wq + bq)(x Wk + bk)^T / 16
         = x (Wq Wk^T / 16) x^T + per-query const (softmax-invariant)
           + per-key term x_k . (Wk bq) (folded into the exp bias)
  so the device computes A = x M (M = 16 Wq Wk^T, sigma~1), scoresT = A x^T
  with x^T stationary, and exp(scoresT/256 + cvec). bv/bo fold into the
  output bias. The softmax division commutes past Wo:
  (num/den) @ Wo = diag(1/den) (num @ Wo), so the final projection runs on
  the unnormalized numerator and the per-query 1/den is applied on the
  output eviction (scalar_tensor_tensor: fp * rec + bias), keeping the
  reciprocal off the tail's critical path.

Precision strategy (rel-err budget 2e-2; this lands ~3e-3):
  - fp8(e4m3) DoubleRow matmuls run 2 rows/cycle with K=256 per pass.
    Every fp8 operand is split hi/lo (lo = fp8 of the residual; operands
    pre-scaled to sigma~1 so residuals clear the subnormal floor) and
    products use the 3-term expansion ah*bh + al*bh + ah*bl: bf16 accuracy
    at 0.75x the bf16 row count. x is split host-side; A on-chip during
    PSUM eviction (ACT copy -> ah, DVE subtract -> al).
  - exp output, PV, denominator sums, final projection: bf16.
  - v is computed as 16 v (Wv pre-scaled); the denominator matmul uses a
    16.0 stationary, so out = (p . 16v) / (16 sum p) exactly.
  - The denominator needs a per-QUERY-partition layout for the fused
    output eviction; a [128,512] broadcast tile is transposed on the PE
    (bf16, 4x128 rows) and reciprocal'd as a [128,4] strided read.

Schedule (single Tile context, PE kept back-to-back):
  - Dummy matmuls over a memset constant warm the PE p-state ramp (to
    2.4 GHz) during the initial DMA wait; packed byte-DMAs carry the
    small weights and x arrives in ordered pieces so A-block 0 starts
    ~3.5 us in (A-block 0 uses the still-idle psacc banks, keeping the
    sc pool free for the V prologue).
  - The scores+exp producer runs LEAD=4 key-tiles ahead of the PV +
    denominator consumer ACROSS query-block boundaries, so the ACT exp
    pipeline never restarts from empty at a boundary.
  - V-projection pairs and later A-projection blocks are interleaved into
    the attention loops (V inside qb0 three pairs ahead of the PV that
    consumes it; A block b+1 inside qb b), so their PSUM evictions hide
    under scores/PV instead of serializing before the loop.
  - Denominator: incremental DVE oct-tree sums (bf16 2x) + one
    16.0-matmul per 8 key-tiles, deferred one group so the PE never
    waits on the tree.
  - Each block's tail (numerator/denominator evictions, den^T transpose,
    reciprocal, final projection, fused-scale eviction, output DMA) is
    software-pipelined into the next block's first iterations; only the
    terminal tail is serial, ordered by dependency readiness.

Per-core PE: A 6.1k + V 12.3k + scores 98.3k + PV 131.1k + denom 6.1k
+ den-transpose 2k + final 8.2k + warmup ~= 266k cycles ~= 111 us at
2.4 GHz (f32r baseline: 343k = 143 us); TimelineSim exec 122.5 us vs
the baseline's 169.2 us. Measured on TRN2: rel err 3.74e-3 (gate 2e-2).
NTFF note: with the toolchain's --enable-ldw-opt=false every matmul
serializes its weight load (unmodeled by the cost model), so raw-HW
exec is LDW-bound for both this kernel (219 us, 256-row DoubleRow
stationaries) and the f32r baseline (208 us); the cost-model metric is
what this kernel optimizes.
"""

import numpy as np
import ml_dtypes

import concourse.bass as bass  # noqa: F401  (AP types come through tile/bacc)
import concourse.tile as tile
from concourse import bacc, mybir
import concourse.bass_utils as _bass_utils
from concourse.bass_utils import run_bass_kernel_spmd

try:  # no S3 in the grading environment; keep artifacts local
    _bass_utils.upload_artifacts = lambda tmpdir: "local://" + tmpdir
except Exception:
    pass

B, S, D = 4, 4096, 256
SQ = S // 2  # queries per core
NCORES = 8
F32 = mybir.dt.float32
BF16 = mybir.dt.bfloat16
F8 = mybir.dt.float8e4
U8 = mybir.dt.uint8
EXP_SCALE = 1.0 / 256.0  # 1/sqrt(D) folded with the 16x M scaling
E4M3 = ml_dtypes.float8_e4m3
DR = mybir.MatmulPerfMode.DoubleRow
ALU = mybir.AluOpType

# wp2b packed byte offsets (per partition)
WP2_WO, WP2_CVEC, WP2_BOB, WP2_IDENT, WP2_END = (0, 1024, 1152, 2176, 2432)


def _build(phases=3):
    nc = bacc.Bacc("TRN2", target_bir_lowering=False, debug=False,
                   num_devices=NCORES)

    # x^T hi/lo splits, chunk-stacked and packed: per partition row d_lo,
    # [xh chunk0 sk | xh chunk1 sk | xl chunk0 sk | xl chunk1 sk] fp8
    xpk_d = nc.dram_tensor("xpk", [128, 4 * S], F8, kind="ExternalInput").ap()
    # wpa: mh|ml|wvh|wvl packed fp8 [d_lo 128, (c 2, e 256)] each
    wpa_d = nc.dram_tensor("wpa", [128, 2048], U8, kind="ExternalInput").ap()
    # wp2b: wo|cvec|bob|ident (see WP2_* offsets)
    wp2b_d = nc.dram_tensor("wp2b", [128, WP2_END], U8,
                            kind="ExternalInput").ap()
    out = nc.dram_tensor("out", [SQ, D], BF16, kind="ExternalOutput").ap()

    out_g = out.rearrange("(g j p) c -> g p j c", j=2, p=128)  # [8,128,2,256]
    xpk_g = xpk_d.rearrange("p (h c s) -> p h c s", h=2, c=2)

    with tile.TileContext(nc) as tc:
        with (
            tc.tile_pool(name="const", bufs=1) as cpool,
            tc.tile_pool(name="pt", bufs=10) as pt_pool,
            tc.tile_pool(name="pts", bufs=8) as pts_pool,
            tc.tile_pool(name="ovec", bufs=2) as ovec_pool,
            tc.tile_pool(name="fout", bufs=2) as fout_pool,
            tc.tile_pool(name="psmm", bufs=1, space="PSUM") as psmm,
            tc.tile_pool(name="psacc", bufs=1, space="PSUM") as psacc,
        ):
            # ---- input DMAs, ordered so A block 0 can start ~3 us in ----
            xpk = cpool.tile([128, 4 * S], F8, tag="xpk", name="xpk")
            xpk_4 = xpk[:].rearrange("p (h c s) -> p h c s", h=2, c=2)
            xht_3 = xpk[:, 0:2 * S].rearrange("p (c s) -> p c s", c=2)
            xlt_3 = xpk[:, 2 * S:4 * S].rearrange("p (c s) -> p c s", c=2)
            wpa = cpool.tile([128, 2048], U8, tag="wpa", name="wpa")
            wp2b = cpool.tile([128, WP2_END], U8, tag="wp2b", name="wp2b")

            nc.sync.dma_start(xpk_4[:, :, :, 0:512], xpk_g[:, :, :, 0:512])
            nc.sync.dma_start(wpa[:], wpa_d)
            nc.sync.dma_start(xpk_4[:, :, :, 512:SQ],
                              xpk_g[:, :, :, 512:SQ])
            nc.sync.dma_start(wp2b[:], wp2b_d)
            nc.sync.dma_start(xpk_4[:, :, :, SQ:S], xpk_g[:, :, :, SQ:S])

            mh = wpa[:, 0:512].bitcast(F8).rearrange("p (c e) -> p c e", c=2)
            ml = wpa[:, 512:1024].bitcast(F8).rearrange(
                "p (c e) -> p c e", c=2)
            wvh = wpa[:, 1024:1536].bitcast(F8).rearrange(
                "p (c e) -> p c e", c=2)
            wvl = wpa[:, 1536:2048].bitcast(F8).rearrange(
                "p (c e) -> p c e", c=2)
            wo_3 = wp2b[:, WP2_WO:WP2_CVEC].bitcast(BF16).rearrange(
                "p (c e) -> p c e", c=2)
            cvec = wp2b[:, WP2_CVEC:WP2_BOB].bitcast(F32)     # [128, 32]
            bob = wp2b[:, WP2_BOB:WP2_IDENT].bitcast(F32)     # [128, 256]
            ident = wp2b[:, WP2_IDENT:WP2_END].bitcast(BF16)  # [128, 128]

            ones16 = cpool.tile([128, 128], BF16, tag="ones16", name="ones16")
            # 16.0 in bf16 is 0x4180
            nc.vector.memset(ones16[:].bitcast(mybir.dt.uint16), 0x4180)

            # ---- persistent activations ----
            ahT = cpool.tile([128, 2 * SQ], F8, tag="ahT", name="ahT")
            alT = cpool.tile([128, 2 * SQ], F8, tag="alT", name="alT")
            ahT_3 = ahT[:].rearrange("p (c q) -> p c q", c=2)
            alT_3 = alT[:].rearrange("p (c q) -> p c q", c=2)
            v_sb = cpool.tile([128, 32 * D], BF16, tag="v", name="v")

            def emit_ablk_et(blk, et, tag="sc"):
                """One e-tile of A^T q-block: 3-term fp8 DoubleRow matmuls
                plus hi/lo split eviction (ACT copy, DVE subtract)."""
                qsl = slice(blk * 512, (blk + 1) * 512)
                esl = slice(et * 128, (et + 1) * 128)
                if tag == "sc":
                    pp = psmm.tile([128, 512], F32, tag="sc", name="sc",
                                   bufs=5)
                else:
                    pp = psacc.tile([128, 512], F32, tag=tag, name=tag)
                nc.tensor.matmul(pp[:], mh[:, :, esl], xht_3[:, :, qsl],
                                 start=True, stop=False, perf_mode=DR)
                nc.tensor.matmul(pp[:], ml[:, :, esl], xht_3[:, :, qsl],
                                 start=False, stop=False, perf_mode=DR)
                nc.tensor.matmul(pp[:], mh[:, :, esl], xlt_3[:, :, qsl],
                                 start=False, stop=True, perf_mode=DR)
                nc.scalar.copy(ahT_3[:, et, qsl], pp[:])
                nc.vector.tensor_sub(alT_3[:, et, qsl], pp[:],
                                     ahT_3[:, et, qsl])

            vparity = [0]

            def emit_vpair(stp, force_dve=False):
                """v16 for sk-tiles 2stp,2stp+1: 3-term fp8 DR, one bank."""
                vp = psmm.tile([128, 512], F32, tag="sc", name="sc", bufs=5)
                for half in range(2):
                    st = stp * 2 + half
                    ssl = slice(st * 128, (st + 1) * 128)
                    osl = slice(half * D, (half + 1) * D)
                    nc.tensor.matmul(vp[:, osl], xht_3[:, :, ssl], wvh[:],
                                     start=True, stop=False, perf_mode=DR)
                    nc.tensor.matmul(vp[:, osl], xlt_3[:, :, ssl], wvh[:],
                                     start=False, stop=False, perf_mode=DR)
                    nc.tensor.matmul(vp[:, osl], xht_3[:, :, ssl], wvl[:],
                                     start=False, stop=True, perf_mode=DR)
                dsl = v_sb[:, stp * 512:(stp + 1) * 512]
                if force_dve or vparity[0] % 2 == 0:
                    nc.vector.tensor_copy(dsl, vp[:])
                else:
                    nc.scalar.copy(dsl, vp[:])
                vparity[0] += 1

            # PE p-state warmup: the ramp to 2.4 GHz needs ~3 us of
            # continuous PE activity; burn the initial DMA wait on dummy
            # matmuls over the memset constant so real work starts at full
            # clock. Results land in the (still unused) accd bank.
            if phases >= 1:
                warm = psacc.tile([128, 512], F32, tag="accd", name="accd")
                for i in range(24):
                    nc.tensor.matmul(warm[:, 0:128], ones16[:],
                                     ones16[:], start=True, stop=True)
                emit_ablk_et(0, 0, tag="accd")
                emit_ablk_et(0, 1, tag="acc0")
                emit_vpair(0)
                emit_vpair(1)
                emit_vpair(2)

            # ---- attention ----
            # The per-block tail (den^T/rec, numerator evictions, final
            # projection) is software-pipelined into the next block's first
            # iterations so its eviction latencies hide under scores/PV.
            def tail_part1(p, terminal=False):
                """Evictions of accd and the numerators; frees all psacc
                banks for the next block. den first: the PE transposes are
                its only consumer and come earliest."""
                den = ovec_pool.tile([128, 512], BF16, tag="den", name="den")
                nc.scalar.copy(den[:], p["accd"][:])
                p["den"] = den
                o0 = ovec_pool.tile([128, 512], BF16, tag="o0", name="o0")
                nc.scalar.copy(o0[:], p["acc"][0][:])
                o1 = ovec_pool.tile([128, 512], BF16, tag="o1", name="o1")
                nc.vector.tensor_copy(o1[:], p["acc"][1][:])
                p["o"] = (o0, o1)

            def tail_part2(p):
                """den^T on the PE (bf16), 1/den as a [128,4] strided read."""
                scd = psmm.tile([128, 512], F32, tag="sc", name="sc", bufs=5)
                scd_bf = scd[:, 0:256].bitcast(BF16)
                den = p["den"]
                for t4 in range(4):
                    nc.tensor.transpose(scd_bf[:, t4 * 128:(t4 + 1) * 128],
                                        den[:, t4 * 128:(t4 + 1) * 128],
                                        ident)
                rec = ovec_pool.tile([128, 4], F32, tag="rec", name="rec")
                nc.vector.reciprocal(
                    rec[:],
                    scd_bf.rearrange("p (b c) -> p b c", c=128)[:, :, 0])
                p["rec"] = rec

            def tail_part3(p):
                """fp = o Wo into the freed accd bank + one sc slot; evict
                with the fused 1/den scale and output bias; DMA out."""
                fp4a = psacc.tile([128, 512], F32, tag="accd", name="accd")
                scx = psmm.tile([128, 512], F32, tag="sc", name="sc", bufs=5)
                fp_slices = [fp4a[:, 0:256], fp4a[:, 256:512],
                             scx[:, 0:256], scx[:, 256:512]]
                o, rec = p["o"], p["rec"]
                for pair in range(2):
                    fo = fout_pool.tile([128, 2 * D], BF16, tag="fout",
                                        name="fout")
                    for half in range(2):
                        t4 = pair * 2 + half
                        tsl = slice(t4 * 128, (t4 + 1) * 128)
                        fp = fp_slices[t4]
                        for e in range(2):
                            nc.tensor.matmul(fp, o[e][:, tsl], wo_3[:, e, :],
                                             start=(e == 0), stop=(e == 1))
                        nc.vector.scalar_tensor_tensor(
                            fo[:, half * D:(half + 1) * D], fp,
                            rec[:, t4:t4 + 1], bob,
                            op0=ALU.mult, op1=ALU.add)
                    nc.sync.dma_start(out_g[p["qb"] * 2 + pair],
                                      fo.rearrange("p (j c) -> p j c", j=2))

            # Producer/consumer skew: scores+exp run LEAD tiles ahead of the
            # PV+denominator consumer, across qb boundaries, so ACT's exp
            # pipeline never restarts from empty when a new block begins.
            LEAD = 4
            state = {"acc": None, "accd": None, "pending": None,
                     "ptq": [], "l1q": [], "l2q": [], "pc_defer": None}

            def consume(cqb, cst, pt):
                if cst == 0:
                    if state["pending"] is not None:
                        tail_part1(state["pending"])
                    state["acc"] = [
                        psacc.tile([128, 512], F32, tag=f"acc{e}",
                                   name=f"acc{e}") for e in range(2)]
                    state["accd"] = None
                acc = state["acc"]
                first, last = (cst == 0), (cst == 31)
                nc.tensor.matmul(acc[0][:], v_sb[:, cst * D:cst * D + 128],
                                 pt[:], start=first, stop=last)
                nc.tensor.matmul(acc[1][:],
                                 v_sb[:, cst * D + 128:(cst + 1) * D],
                                 pt[:], start=first, stop=last)
                p = state["pending"]
                if p is not None:
                    if cst == 2:
                        tail_part2(p)
                    elif cst == 3:
                        tail_part3(p)
                        state["pending"] = None
                # Denominator: incremental oct-tree sums on DVE (bf16 2x
                # mode) as pairs complete, then one 16.0-stationary matmul
                # per 8 tiles.
                ptq, l1q, l2q = state["ptq"], state["l1q"], state["l2q"]
                ptq.append(pt)
                if cst % 2 == 1:
                    ps = pts_pool.tile([128, 512], BF16, tag="pts",
                                       name="pts")
                    nc.vector.tensor_add(ps[:], ptq[-2][:], ptq[-1][:])
                    l1q.append(ps)
                if cst % 8 == 3 or cst % 8 == 5:
                    # running fold: covers tiles 0..3, then 0..5
                    ps = pts_pool.tile([128, 512], BF16, tag="pts",
                                       name="pts")
                    nc.vector.tensor_add(ps[:], l1q[-2][:] if cst % 8 == 3
                                         else l2q[-1][:], l1q[-1][:])
                    l2q.append(ps)
                if cst % 8 == 7:
                    ps = pts_pool.tile([128, 512], BF16, tag="pts",
                                       name="pts")
                    nc.vector.tensor_add(ps[:], l2q[-1][:], l1q[-1][:])
                    if state["accd"] is None:
                        state["accd"] = psacc.tile([128, 512], F32,
                                                   tag="accd", name="accd")
                    if cst == 31:
                        if state["pc_defer"] is not None:
                            nc.tensor.matmul(state["accd"][:], ones16[:],
                                             state["pc_defer"][:],
                                             start=False, stop=False)
                        nc.tensor.matmul(state["accd"][:], ones16[:], ps[:],
                                         start=False, stop=True)
                        state["pc_defer"] = None
                    else:
                        if state["pc_defer"] is not None:
                            nc.tensor.matmul(state["accd"][:], ones16[:],
                                             state["pc_defer"][:],
                                             start=(cst == 15), stop=False)
                        state["pc_defer"] = ps
                    state["ptq"], state["l1q"], state["l2q"] = [], [], []
                if cst == 31:
                    state["pending"] = {"acc": acc, "accd": state["accd"],
                                        "qb": cqb}

            nqb = SQ // 512 if phases >= 2 else 0
            fifo = []
            for qb in range(nqb):
                qsl = slice(qb * 512, (qb + 1) * 512)
                for st in range(32):
                    # interleaved producer work for later consumers
                    if qb == 0 and st % 2 == 0 and st // 2 + 3 <= 15:
                        emit_vpair(st // 2 + 3)
                    if qb < 3 and st in (11, 21):
                        emit_ablk_et(qb + 1, 0 if st == 11 else 1)

                    ssl = slice(st * 128, (st + 1) * 128)
                    sp = psmm.tile([128, 512], F32, tag="sc", name="sc",
                                   bufs=5)
                    nc.tensor.matmul(sp[:], xht_3[:, :, ssl],
                                     ahT_3[:, :, qsl], start=True, stop=False,
                                     perf_mode=DR)
                    nc.tensor.matmul(sp[:], xht_3[:, :, ssl],
                                     alT_3[:, :, qsl], start=False,
                                     stop=False, perf_mode=DR)
                    nc.tensor.matmul(sp[:], xlt_3[:, :, ssl],
                                     ahT_3[:, :, qsl], start=False, stop=True,
                                     perf_mode=DR)
                    pt = pt_pool.tile([128, 512], BF16, tag="pt", name="pt",
                                      bufs=10)
                    nc.scalar.activation(pt[:], sp[:],
                                         mybir.ActivationFunctionType.Exp,
                                         scale=EXP_SCALE,
                                         bias=cvec[:, st:st + 1])
                    fifo.append((qb, st, pt))
                    if len(fifo) > LEAD:
                        consume(*fifo.pop(0))
            for item in fifo:
                consume(*item)

            if state["pending"] is not None and phases >= 2:
                # Terminal tail: nothing left to overlap with, so order by
                # dependency readiness: numerators evict first (fp matmuls
                # need only those), the denominator chain runs concurrently,
                # and each output row-tile DMAs as soon as it is scaled.
                p = state["pending"]
                o0 = ovec_pool.tile([128, 512], BF16, tag="o0", name="o0")
                nc.scalar.copy(o0[:], p["acc"][0][:])
                o1 = ovec_pool.tile([128, 512], BF16, tag="o1", name="o1")
                nc.vector.tensor_copy(o1[:], p["acc"][1][:])
                o = (o0, o1)
                den = ovec_pool.tile([128, 512], BF16, tag="den", name="den")
                nc.scalar.copy(den[:], p["accd"][:])
                fpa = psacc.tile([128, 512], F32, tag="acc0",
                                 name="acc0")
                fpb = psacc.tile([128, 512], F32, tag="acc1",
                                 name="acc1")
                fp_slices = [fpa[:, 0:256], fpa[:, 256:512],
                             fpb[:, 0:256], fpb[:, 256:512]]
                for t4 in range(4):
                    tsl = slice(t4 * 128, (t4 + 1) * 128)
                    for e in range(2):
                        nc.tensor.matmul(fp_slices[t4], o[e][:, tsl],
                                         wo_3[:, e, :],
                                         start=(e == 0), stop=(e == 1))
                scd = psmm.tile([128, 512], F32, tag="sc", name="sc", bufs=5)
                scd_bf = scd[:, 0:256].bitcast(BF16)
                for t4 in range(4):
                    nc.tensor.transpose(scd_bf[:, t4 * 128:(t4 + 1) * 128],
                                        den[:, t4 * 128:(t4 + 1) * 128],
                                        ident)
                rec = ovec_pool.tile([128, 4], F32, tag="rec", name="rec")
                nc.vector.reciprocal(
                    rec[:],
                    scd_bf.rearrange("p (b c) -> p b c", c=128)[:, :, 0])
                for pair in range(2):
                    fo = fout_pool.tile([128, 2 * D], BF16, tag="fout",
                                        name="fout")
                    for half in range(2):
                        t4 = pair * 2 + half
                        nc.vector.scalar_tensor_tensor(
                            fo[:, half * D:(half + 1) * D], fp_slices[t4],
                            rec[:, t4:t4 + 1], bob,
                            op0=ALU.mult, op1=ALU.add)
                    nc.sync.dma_start(out_g[p["qb"] * 2 + pair],
                                      fo.rearrange("p (j c) -> p j c", j=2))

    nc.compile()
    return nc


_NC = None


def _get_nc():
    global _NC
    if _NC is None:
        _NC = _build()
    return _NC


class _Runner:
    """Cached jitted SPMD executor (run_bass_kernel_spmd rebuilds its jax
    closure every call, forcing a retrace; this traces once)."""

    def __init__(self, nc):
        import jax
        from jax.sharding import Mesh, PartitionSpec
        from jax.experimental.shard_map import shard_map
        from concourse import bass2jax, mybir as mb

        bass2jax.install_neuronx_cc_hook()
        self.jax = jax
        if not any("axon" in str(getattr(d, "platform", "")).lower()
                   or str(d).startswith("NC_")
                   for d in jax.devices()):
            # jax was initialized on another platform (e.g. cpu for the
            # reference); reset so the axon NeuronCores are visible.
            import jax._src.xla_bridge as xb
            jax.config.update("jax_platforms", None)
            xb._clear_backends()
            if hasattr(xb.get_backend, "cache_clear"):
                xb.get_backend.cache_clear()
            if not any("axon" in str(getattr(d, "platform", "")).lower()
                       or str(d).startswith("NC_")
                       for d in jax.devices()):
                jax.config.update("jax_platforms", "axon")
                xb._clear_backends()
                if hasattr(xb.get_backend, "cache_clear"):
                    xb.get_backend.cache_clear()
        partition_name = (nc.partition_id_tensor.name
                          if nc.partition_id_tensor else None)
        in_names, out_names, out_avals = [], [], []
        for alloc in nc.m.functions[0].allocations:
            if not isinstance(alloc, mb.MemoryLocationSet):
                continue
            name = alloc.memorylocations[0].name
            if alloc.kind == "ExternalInput":
                if name != partition_name:
                    in_names.append(name)
            elif alloc.kind == "ExternalOutput":
                out_names.append(name)
                out_avals.append(jax.core.ShapedArray(
                    tuple(alloc.tensor_shape), mb.dt.np(alloc.dtype)))
        self.in_names, self.out_names, self.out_avals = \
            in_names, out_names, out_avals
        n_params, n_outs = len(in_names), len(out_names)
        bind_in_names = in_names + out_names + (
            [partition_name] if partition_name else [])

        def _body(*args):
            operands = list(args)
            if partition_name is not None:
                operands.append(bass2jax.partition_id_tensor())
            outs = bass2jax._bass_exec_p.bind(
                *operands,
                out_avals=tuple(out_avals),
                in_names=tuple(bind_in_names),
                out_names=tuple(out_names),
                lowering_input_output_aliases=(),
                sim_require_finite=True,
                sim_require_nnan=True,
                nc=nc,
            )
            return tuple(outs)

        devices = jax.devices()[:NCORES]
        mesh = Mesh(np.asarray(devices), ("core",))
        spec = (PartitionSpec("core"),) * (n_params + n_outs)
        self.fn = jax.jit(
            shard_map(_body, mesh=mesh, in_specs=spec,
                      out_specs=(PartitionSpec("core"),) * n_outs,
                      check_rep=False),
            donate_argnums=tuple(range(n_params, n_params + n_outs)),
            keep_unused=True,
        )

    def run(self, in_maps):
        concat_in = [
            np.concatenate([np.asarray(m[n]) for m in in_maps], axis=0)
            for n in self.in_names
        ]
        concat_zeros = [
            np.zeros((NCORES * a.shape[0], *a.shape[1:]), a.dtype)
            for a in self.out_avals
        ]
        outs = self.fn(*concat_in, *concat_zeros)
        return [
            {n: np.asarray(outs[i]).reshape(NCORES, *self.out_avals[i].shape)[c]
             for i, n in enumerate(self.out_names)}
            for c in range(NCORES)
        ]


_RUNNER = None


def _get_runner():
    global _RUNNER
    if _RUNNER is None:
        _RUNNER = _Runner(_get_nc())
    return _RUNNER


def _split8(a):
    """fp8 e4m3 hi/lo split: a ~= hi + lo elementwise."""
    hi = np.asarray(a, dtype=E4M3)
    lo = np.asarray(a.astype(np.float32) - hi.astype(np.float32), dtype=E4M3)
    return hi, lo


def _dstack(a):
    """[256, N] (d-major) -> [128, 2*N] chunk-stacked: row d_lo holds
    (chunk 0 cols, chunk 1 cols)."""
    n = a.shape[1]
    return np.ascontiguousarray(
        a.reshape(2, 128, n).transpose(1, 0, 2).reshape(128, 2 * n))


def make_in_maps(inputs):
    x = np.asarray(inputs["x"], dtype=np.float32)
    Wq = np.asarray(inputs["Wq"], dtype=np.float32)
    Wk = np.asarray(inputs["Wk"], dtype=np.float32)
    Wv = np.asarray(inputs["Wv"], dtype=np.float32)
    Wo = np.asarray(inputs["Wo"], dtype=np.float32)
    bq = np.asarray(inputs["bq"], dtype=np.float32)
    bv = np.asarray(inputs["bv"], dtype=np.float32)
    bo = np.asarray(inputs["bo"], dtype=np.float32)
    # bk drops out of softmax (per-query constant). bq only survives through
    # the per-key term x_k . (Wk bq), applied as an exp bias. bv folds into
    # the output bias (attention rows sum to 1).
    u8 = np.uint8
    M16 = (16.0 * (Wq @ Wk.T)).astype(np.float32)
    mh, ml = _split8(M16)
    wvh, wvl = _split8((16.0 * Wv).astype(np.float32))
    wpa = np.ascontiguousarray(np.concatenate(
        [_dstack(mh).view(u8), _dstack(ml).view(u8),
         _dstack(wvh).view(u8), _dstack(wvl).view(u8)], axis=1))
    wo_b = _dstack(np.asarray(Wo, dtype=ml_dtypes.bfloat16))
    bob = np.tile((bv @ Wo + bo).astype(np.float32)[None, :], (128, 1))
    ident = np.eye(128, dtype=ml_dtypes.bfloat16)
    wkbq = (Wk @ bq).astype(np.float32)
    in_maps = []
    for c in range(NCORES):
        b, h = divmod(c, 2)
        # Rotate the batch so this core's queries are rows 0..SQ-1; keys and
        # values see all rows either way (softmax is key-order invariant).
        xb = x[b] if h == 0 else np.ascontiguousarray(
            np.concatenate([x[b, SQ:], x[b, :SQ]]))
        xh, xl = _split8(xb)
        cvec = np.ascontiguousarray(
            (xb @ wkbq).astype(np.float32).reshape(32, 128).T)
        wp2b = np.ascontiguousarray(np.concatenate(
            [wo_b.view(u8), cvec.view(u8), bob.view(u8), ident.view(u8)],
            axis=1))
        assert wp2b.shape == (128, WP2_END)
        in_maps.append({
            "xpk": np.concatenate(
                [_dstack(np.ascontiguousarray(xh.T)),
                 _dstack(np.ascontiguousarray(xl.T))], axis=1),
            "wpa": wpa,
            "wp2b": wp2b,
        })
    return in_maps


def kernel(**inputs):
    try:
        runner = _get_runner()
    except Exception:
        runner = None
    in_maps = make_in_maps(inputs)
    results = None
    if runner is not None:
        try:
            results = runner.run(in_maps)
        except Exception:
            results = None
    if results is None:
        results = run_bass_kernel_spmd(
            _get_nc(), in_maps, core_ids=list(range(NCORES))).results
    outp = np.empty((B, S, D), dtype=np.float32)
    for c in range(NCORES):
        b, h = divmod(c, 2)
        outp[b, h * SQ:(h + 1) * SQ] = \
            results[c]["out"].astype(np.float32)
    return outp


# revision 88
# speedup vs baseline: 1.0007x; 1.0007x over previous
"""Trainium2 Bass kernel: single-head attention module (dense transformer).

Computes, for x [4, 4096, 256] (f32) and per-projection weights/biases:
    q = x @ Wq + bq;  k = x @ Wk + bk;  v = x @ Wv + bv
    out = softmax((q k^T) / sqrt(256)) @ v @ Wo + bo

Sharding over 8 NeuronCores: core c handles batch c//2, query half c%2.
The host rotates each core's batch so its queries are always rows 0..2047
(softmax is key-order invariant), keeping the device program identical
across cores.

Math rewrite (host-side, weights only):
  scores = (x Wq + bq)(x Wk + bk)^T / 16
         = x (Wq Wk^T / 16) x^T + per-query const (softmax-invariant)
           + per-key term x_k . (Wk bq) (folded into the exp bias)
  so the device computes A = x M (M = 16 Wq Wk^T, sigma~1), scoresT = A x^T
  with x^T stationary, and exp(scoresT/256 + cvec). bv/bo fold into the
  output bias. The softmax division commutes past Wo:
  (num/den) @ Wo = diag(1/den) (num @ Wo), so the final projection runs on
  the unnormalized numerator and the per-query 1/den is applied on the
  output eviction (scalar_tensor_tensor: fp * rec + bias), keeping the
  reciprocal off the tail's critical path.

Precision strategy (rel-err budget 2e-2; this lands ~3e-3):
  - fp8(e4m3) DoubleRow matmuls run 2 rows/cycle with K=256 per pass.
    Every fp8 operand is split hi/lo (lo = fp8 of the residual; operands
    pre-scaled to sigma~1 so residuals clear the subnormal floor) and
    products use the 3-term expansion ah*bh + al*bh + ah*bl: bf16 accuracy
    at 0.75x the bf16 row count. x is split host-side; A on-chip during
    PSUM eviction (ACT copy -> ah, DVE subtract -> al).
  - exp output, PV, denominator sums, final projection: bf16.
  - v is computed as 16 v (Wv pre-scaled); the denominator matmul uses a
    16.0 stationary, so out = (p . 16v) / (16 sum p) exactly.
  - The denominator needs a per-QUERY-partition layout for the fused
    output eviction; a [128,512] broadcast tile is transposed on the PE
    (bf16, 4x128 rows) and reciprocal'd as a [128,4] strided read.

Schedule (single Tile context, PE kept back-to-back):
  - Dummy matmuls over a memset constant warm the PE p-state ramp (to
    2.4 GHz) during the initial DMA wait; packed byte-DMAs carry the
    small weights and x arrives in ordered pieces so A-block 0 starts
    ~3.5 us in (A-block 0 uses the still-idle psacc banks, keeping the
    sc pool free for the V prologue).
  - The scores+exp producer runs LEAD=4 key-tiles ahead of the PV +
    denominator consumer ACROSS query-block boundaries, so the ACT exp
    pipeline never restarts from empty at a boundary.
  - V-projection pairs and later A-projection blocks are interleaved into
    the attention loops (V inside qb0 three pairs ahead of the PV that
    consumes it; A block b+1 inside qb b), so their PSUM evictions hide
    under scores/PV instead of serializing before the loop.
  - Denominator: incremental DVE oct-tree sums (bf16 2x) + one
    16.0-matmul per 8 key-tiles, deferred one group so the PE never
    waits on the tree.
  - Each block's tail (numerator/denominator evictions, den^T transpose,
    reciprocal, final projection, fused-scale eviction, output DMA) is
    software-pipelined into the next block's first iterations; only the
    terminal tail is serial, ordered by dependency readiness.

Per-core PE: A 6.1k + V 12.3k + scores 98.3k + PV 131.1k + denom 6.1k
+ den-transpose 2k + final 8.2k + warmup ~= 266k cycles ~= 111 us at
2.4 GHz (f32r baseline: 343k = 143 us); TimelineSim exec 122.5 us vs
the baseline's 169.2 us. Measured on TRN2: rel err 3.74e-3 (gate 2e-2).
NTFF note: with the toolchain's --enable-ldw-opt=false every matmul
serializes its weight load (unmodeled by the cost model), so raw-HW
exec is LDW-bound for both this kernel (219 us, 256-row DoubleRow
stationaries) and the f32r baseline (208 us); the cost-model metric is
what this kernel optimizes.
"""

import numpy as np
import ml_dtypes

import concourse.bass as bass  # noqa: F401  (AP types come through tile/bacc)
import concourse.tile as tile
from concourse import bacc, mybir
import concourse.bass_utils as _bass_utils
from concourse.bass_utils import run_bass_kernel_spmd

try:  # no S3 in the grading environment; keep artifacts local
    _bass_utils.upload_artifacts = lambda tmpdir: "local://" + tmpdir
except Exception:
    pass

B, S, D = 4, 4096, 256
SQ = S // 2  # queries per core
NCORES = 8
F32 = mybir.dt.float32
BF16 = mybir.dt.bfloat16
F8 = mybir.dt.float8e4
U8 = mybir.dt.uint8
EXP_SCALE = 1.0 / 256.0  # 1/sqrt(D) folded with the 16x M scaling
E4M3 = ml_dtypes.float8_e4m3
DR = mybir.MatmulPerfMode.DoubleRow
ALU = mybir.AluOpType

# wp2b packed byte offsets (per partition)
WP2_WO, WP2_CVEC, WP2_BOB, WP2_IDENT, WP2_END = (0, 1024, 1152, 2176, 2432)


def _build(phases=3, has_bias=False):
    nc = bacc.Bacc("TRN2", target_bir_lowering=False, debug=False,
                   num_devices=NCORES)

    # x^T hi/lo splits, chunk-stacked and packed: per partition row d_lo,
    # [xh chunk0 sk | xh chunk1 sk | xl chunk0 sk | xl chunk1 sk] fp8
    xpk_d = nc.dram_tensor("xpk", [128, 4 * S], F8, kind="ExternalInput").ap()
    # wpa: mh|ml|wvh|wvl packed fp8 [d_lo 128, (c 2, e 256)] each
    wpa_d = nc.dram_tensor("wpa", [128, 2048], U8, kind="ExternalInput").ap()
    # wp2b: wo|cvec|bob|ident (see WP2_* offsets)
    wp2b_d = nc.dram_tensor("wp2b", [128, WP2_END], U8,
                            kind="ExternalInput").ap()
    out = nc.dram_tensor("out", [SQ, D], BF16, kind="ExternalOutput").ap()

    out_g = out.rearrange("(g j p) c -> g p j c", j=2, p=128)  # [8,128,2,256]
    xpk_g = xpk_d.rearrange("p (h c s) -> p h c s", h=2, c=2)

    with tile.TileContext(nc) as tc:
        with (
            tc.tile_pool(name="const", bufs=1) as cpool,
            tc.tile_pool(name="pt", bufs=10) as pt_pool,
            tc.tile_pool(name="pts", bufs=8) as pts_pool,
            tc.tile_pool(name="ovec", bufs=2) as ovec_pool,
            tc.tile_pool(name="fout", bufs=2) as fout_pool,
            tc.tile_pool(name="psmm", bufs=1, space="PSUM") as psmm,
            tc.tile_pool(name="psacc", bufs=1, space="PSUM") as psacc,
        ):
            # ---- input DMAs, ordered so A block 0 can start ~3 us in ----
            xpk = cpool.tile([128, 4 * S], F8, tag="xpk", name="xpk")
            xpk_4 = xpk[:].rearrange("p (h c s) -> p h c s", h=2, c=2)
            xht_3 = xpk[:, 0:2 * S].rearrange("p (c s) -> p c s", c=2)
            xlt_3 = xpk[:, 2 * S:4 * S].rearrange("p (c s) -> p c s", c=2)
            wpa = cpool.tile([128, 2048], U8, tag="wpa", name="wpa")
            wp2b = cpool.tile([128, WP2_END], U8, tag="wp2b", name="wp2b")

            nc.sync.dma_start(xpk_4[:, :, :, 0:512], xpk_g[:, :, :, 0:512])
            nc.sync.dma_start(wpa[:], wpa_d)
            nc.sync.dma_start(xpk_4[:, :, :, 512:SQ],
                              xpk_g[:, :, :, 512:SQ])
            nc.sync.dma_start(wp2b[:], wp2b_d)
            nc.sync.dma_start(xpk_4[:, :, :, SQ:S], xpk_g[:, :, :, SQ:S])

            mh = wpa[:, 0:512].bitcast(F8).rearrange("p (c e) -> p c e", c=2)
            ml = wpa[:, 512:1024].bitcast(F8).rearrange(
                "p (c e) -> p c e", c=2)
            wvh = wpa[:, 1024:1536].bitcast(F8).rearrange(
                "p (c e) -> p c e", c=2)
            wvl = wpa[:, 1536:2048].bitcast(F8).rearrange(
                "p (c e) -> p c e", c=2)
            wo_3 = wp2b[:, WP2_WO:WP2_CVEC].bitcast(BF16).rearrange(
                "p (c e) -> p c e", c=2)
            cvec = wp2b[:, WP2_CVEC:WP2_BOB].bitcast(F32)     # [128, 32]
            bob = wp2b[:, WP2_BOB:WP2_IDENT].bitcast(F32)     # [128, 256]
            ident = wp2b[:, WP2_IDENT:WP2_END].bitcast(BF16)  # [128, 128]

            ones16 = cpool.tile([128, 128], BF16, tag="ones16", name="ones16")
            # 16.0 in bf16 is 0x4180
            nc.vector.memset(ones16[:].bitcast(mybir.dt.uint16), 0x4180)

            # ---- persistent activations ----
            ahT = cpool.tile([128, 2 * SQ], F8, tag="ahT", name="ahT")
            alT = cpool.tile([128, 2 * SQ], F8, tag="alT", name="alT")
            ahT_3 = ahT[:].rearrange("p (c q) -> p c q", c=2)
            alT_3 = alT[:].rearrange("p (c q) -> p c q", c=2)
            v_sb = cpool.tile([128, 32 * D], BF16, tag="v", name="v")

            def emit_ablk_et(blk, et, tag="sc"):
                """One e-tile of A^T q-block: 3-term fp8 DoubleRow matmuls
                plus hi/lo split eviction (ACT copy, DVE subtract)."""
                qsl = slice(blk * 512, (blk + 1) * 512)
                esl = slice(et * 128, (et + 1) * 128)
                if tag == "sc":
                    pp = psmm.tile([128, 512], F32, tag="sc", name="sc",
                                   bufs=5)
                else:
                    pp = psacc.tile([128, 512], F32, tag=tag, name=tag)
                nc.tensor.matmul(pp[:], mh[:, :, esl], xht_3[:, :, qsl],
                                 start=True, stop=False, perf_mode=DR)
                nc.tensor.matmul(pp[:], ml[:, :, esl], xht_3[:, :, qsl],
                                 start=False, stop=False, perf_mode=DR)
                nc.tensor.matmul(pp[:], mh[:, :, esl], xlt_3[:, :, qsl],
                                 start=False, stop=True, perf_mode=DR)
                nc.scalar.copy(ahT_3[:, et, qsl], pp[:])
                nc.vector.tensor_sub(alT_3[:, et, qsl], pp[:],
                                     ahT_3[:, et, qsl])

            vparity = [0]

            def emit_vpair(stp, force_dve=False):
                """v16 for sk-tiles 2stp,2stp+1: 3-term fp8 DR, one bank."""
                vp = psmm.tile([128, 512], F32, tag="sc", name="sc", bufs=5)
                for half in range(2):
                    st = stp * 2 + half
                    ssl = slice(st * 128, (st + 1) * 128)
                    osl = slice(half * D, (half + 1) * D)
                    nc.tensor.matmul(vp[:, osl], xht_3[:, :, ssl], wvh[:],
                                     start=True, stop=False, perf_mode=DR)
                    nc.tensor.matmul(vp[:, osl], xlt_3[:, :, ssl], wvh[:],
                                     start=False, stop=False, perf_mode=DR)
                    nc.tensor.matmul(vp[:, osl], xht_3[:, :, ssl], wvl[:],
                                     start=False, stop=True, perf_mode=DR)
                dsl = v_sb[:, stp * 512:(stp + 1) * 512]
                if force_dve or vparity[0] % 2 == 0:
                    nc.vector.tensor_copy(dsl, vp[:])
                else:
                    nc.scalar.copy(dsl, vp[:])
                vparity[0] += 1

            # PE p-state warmup: the ramp to 2.4 GHz needs ~3 us of
            # continuous PE activity; burn the initial DMA wait on dummy
            # matmuls over the memset constant so real work starts at full
            # clock. Results land in the (still unused) accd bank.
            if phases >= 1:
                warm = psacc.tile([128, 512], F32, tag="accd", name="accd")
                for i in range(24):
                    nc.tensor.matmul(warm[:, 0:128], ones16[:],
                                     ones16[:], start=True, stop=True)
                emit_ablk_et(0, 0, tag="accd")
                emit_ablk_et(0, 1, tag="acc0")
                emit_vpair(0)
                emit_vpair(1)
                emit_vpair(2)

            # ---- attention ----
            # The per-block tail (den^T/rec, numerator evictions, final
            # projection) is software-pipelined into the next block's first
            # iterations so its eviction latencies hide under scores/PV.
            def tail_part1(p, terminal=False):
                """Evictions of accd and the numerators; frees all psacc
                banks for the next block. den first: the PE transposes are
                its only consumer and come earliest."""
                den = ovec_pool.tile([128, 512], BF16, tag="den", name="den")
                nc.scalar.copy(den[:], p["accd"][:])
                p["den"] = den
                o0 = ovec_pool.tile([128, 512], BF16, tag="o0", name="o0")
                nc.scalar.copy(o0[:], p["acc"][0][:])
                o1 = ovec_pool.tile([128, 512], BF16, tag="o1", name="o1")
                nc.vector.tensor_copy(o1[:], p["acc"][1][:])
                p["o"] = (o0, o1)

            def tail_part2(p):
                """den^T on the PE (bf16), 1/den as a [128,4] strided read."""
                scd = psmm.tile([128, 512], F32, tag="sc", name="sc", bufs=5)
                scd_bf = scd[:, 0:256].bitcast(BF16)
                den = p["den"]
                for t4 in range(4):
                    nc.tensor.transpose(scd_bf[:, t4 * 128:(t4 + 1) * 128],
                                        den[:, t4 * 128:(t4 + 1) * 128],
                                        ident)
                rec = ovec_pool.tile([128, 4], F32, tag="rec", name="rec")
                nc.vector.reciprocal(
                    rec[:],
                    scd_bf.rearrange("p (b c) -> p b c", c=128)[:, :, 0])
                p["rec"] = rec

            def tail_part3(p):
                """fp = o Wo into the freed accd bank + one sc slot; evict
                with the fused 1/den scale and output bias; DMA out."""
                fp4a = psacc.tile([128, 512], F32, tag="accd", name="accd")
                scx = psmm.tile([128, 512], F32, tag="sc", name="sc", bufs=5)
                fp_slices = [fp4a[:, 0:256], fp4a[:, 256:512],
                             scx[:, 0:256], scx[:, 256:512]]
                o, rec = p["o"], p["rec"]
                for pair in range(2):
                    fo = fout_pool.tile([128, 2 * D], BF16, tag="fout",
                                        name="fout")
                    for half in range(2):
                        t4 = pair * 2 + half
                        tsl = slice(t4 * 128, (t4 + 1) * 128)
                        fp = fp_slices[t4]
                        for e in range(2):
                            nc.tensor.matmul(fp, o[e][:, tsl], wo_3[:, e, :],
                                             start=(e == 0), stop=(e == 1))
                        fsl = fo[:, half * D:(half + 1) * D]
                        if has_bias:
                            nc.vector.scalar_tensor_tensor(
                                fsl, fp, rec[:, t4:t4 + 1], bob,
                                op0=ALU.mult, op1=ALU.add)
                        elif half == 0:
                            nc.vector.tensor_scalar_mul(fsl, fp,
                                                        rec[:, t4:t4 + 1])
                        else:
                            nc.scalar.mul(fsl, fp, rec[:, t4:t4 + 1])
                    nc.sync.dma_start(out_g[p["qb"] * 2 + pair],
                                      fo.rearrange("p (j c) -> p j c", j=2))

            # Producer/consumer skew: scores+exp run LEAD tiles ahead of the
            # PV+denominator consumer, across qb boundaries, so ACT's exp
            # pipeline never restarts from empty when a new block begins.
            LEAD = 4
            state = {"acc": None, "accd": None, "pending": None,
                     "ptq": [], "l1q": [], "l2q": [], "pc_defer": None}

            def consume(cqb, cst, pt):
                if cst == 0:
                    if state["pending"] is not None:
                        tail_part1(state["pending"])
                    state["acc"] = [
                        psacc.tile([128, 512], F32, tag=f"acc{e}",
                                   name=f"acc{e}") for e in range(2)]
                    state["accd"] = None
                acc = state["acc"]
                first, last = (cst == 0), (cst == 31)
                nc.tensor.matmul(acc[0][:], v_sb[:, cst * D:cst * D + 128],
                                 pt[:], start=first, stop=last)
                nc.tensor.matmul(acc[1][:],
                                 v_sb[:, cst * D + 128:(cst + 1) * D],
                                 pt[:], start=first, stop=last)
                p = state["pending"]
                if p is not None:
                    if cst == 2:
                        tail_part2(p)
                    elif cst == 3:
                        tail_part3(p)
                        state["pending"] = None
                # Denominator: incremental oct-tree sums on DVE (bf16 2x
                # mode) as pairs complete, then one 16.0-stationary matmul
                # per 8 tiles.
                ptq, l1q, l2q = state["ptq"], state["l1q"], state["l2q"]
                ptq.append(pt)
                if cst % 2 == 1:
                    ps = pts_pool.tile([128, 512], BF16, tag="pts",
                                       name="pts")
                    nc.vector.tensor_add(ps[:], ptq[-2][:], ptq[-1][:])
                    l1q.append(ps)
                if cst % 8 == 3 or cst % 8 == 5:
                    # running fold: covers tiles 0..3, then 0..5
                    ps = pts_pool.tile([128, 512], BF16, tag="pts",
                                       name="pts")
                    nc.vector.tensor_add(ps[:], l1q[-2][:] if cst % 8 == 3
                                         else l2q[-1][:], l1q[-1][:])
                    l2q.append(ps)
                if cst % 8 == 7:
                    ps = pts_pool.tile([128, 512], BF16, tag="pts",
                                       name="pts")
                    nc.vector.tensor_add(ps[:], l2q[-1][:], l1q[-1][:])
                    if state["accd"] is None:
                        state["accd"] = psacc.tile([128, 512], F32,
                                                   tag="accd", name="accd")
                    if cst == 31:
                        if state["pc_defer"] is not None:
                            nc.tensor.matmul(state["accd"][:], ones16[:],
                                             state["pc_defer"][:],
                                             start=False, stop=False)
                        nc.tensor.matmul(state["accd"][:], ones16[:], ps[:],
                                         start=False, stop=True)
                        state["pc_defer"] = None
                    else:
                        if state["pc_defer"] is not None:
                            nc.tensor.matmul(state["accd"][:], ones16[:],
                                             state["pc_defer"][:],
                                             start=(cst == 15), stop=False)
                        state["pc_defer"] = ps
                    state["ptq"], state["l1q"], state["l2q"] = [], [], []
                if cst == 31:
                    state["pending"] = {"acc": acc, "accd": state["accd"],
                                        "qb": cqb}

            nqb = SQ // 512 if phases >= 2 else 0
            fifo = []
            for qb in range(nqb):
                qsl = slice(qb * 512, (qb + 1) * 512)
                for st in range(32):
                    # interleaved producer work for later consumers
                    if qb == 0 and st % 2 == 0 and st // 2 + 7 <= 15:
                        emit_vpair(st // 2 + 7)
                    if qb < 3 and st in (11, 21):
                        emit_ablk_et(qb + 1, 0 if st == 11 else 1)

                    ssl = slice(st * 128, (st + 1) * 128)
                    sp = psmm.tile([128, 512], F32, tag="sc", name="sc",
                                   bufs=5)
                    nc.tensor.matmul(sp[:], xht_3[:, :, ssl],
                                     ahT_3[:, :, qsl], start=True, stop=False,
                                     perf_mode=DR)
                    nc.tensor.matmul(sp[:], xht_3[:, :, ssl],
                                     alT_3[:, :, qsl], start=False,
                                     stop=False, perf_mode=DR)
                    nc.tensor.matmul(sp[:], xlt_3[:, :, ssl],
                                     ahT_3[:, :, qsl], start=False, stop=True,
                                     perf_mode=DR)
                    pt = pt_pool.tile([128, 512], BF16, tag="pt", name="pt",
                                      bufs=10)
                    nc.scalar.activation(pt[:], sp[:],
                                         mybir.ActivationFunctionType.Exp,
                                         scale=EXP_SCALE,
                                         bias=cvec[:, st:st + 1])
                    fifo.append((qb, st, pt))
                    if len(fifo) > LEAD:
                        consume(*fifo.pop(0))
            for item in fifo:
                consume(*item)

            if state["pending"] is not None and phases >= 2:
                # Terminal tail: nothing left to overlap with, so order by
                # dependency readiness: numerators evict first (fp matmuls
                # need only those), the denominator chain runs concurrently,
                # and each output row-tile DMAs as soon as it is scaled.
                p = state["pending"]
                o0 = ovec_pool.tile([128, 512], BF16, tag="o0", name="o0")
                nc.scalar.copy(o0[:], p["acc"][0][:])
                o1 = ovec_pool.tile([128, 512], BF16, tag="o1", name="o1")
                nc.vector.tensor_copy(o1[:], p["acc"][1][:])
                o = (o0, o1)
                den = ovec_pool.tile([128, 512], BF16, tag="den", name="den")
                nc.scalar.copy(den[:], p["accd"][:])
                fpa = psacc.tile([128, 512], F32, tag="acc0",
                                 name="acc0")
                fpb = psacc.tile([128, 512], F32, tag="acc1",
                                 name="acc1")
                fp_slices = [fpa[:, 0:256], fpa[:, 256:512],
                             fpb[:, 0:256], fpb[:, 256:512]]
                for t4 in range(4):
                    tsl = slice(t4 * 128, (t4 + 1) * 128)
                    for e in range(2):
                        nc.tensor.matmul(fp_slices[t4], o[e][:, tsl],
                                         wo_3[:, e, :],
                                         start=(e == 0), stop=(e == 1))
                scd = psmm.tile([128, 512], F32, tag="sc", name="sc", bufs=5)
                scd_bf = scd[:, 0:256].bitcast(BF16)
                for t4 in range(4):
                    nc.tensor.transpose(scd_bf[:, t4 * 128:(t4 + 1) * 128],
                                        den[:, t4 * 128:(t4 + 1) * 128],
                                        ident)
                rec = ovec_pool.tile([128, 4], F32, tag="rec", name="rec")
                nc.vector.reciprocal(
                    rec[:],
                    scd_bf.rearrange("p (b c) -> p b c", c=128)[:, :, 0])
                for pair in range(2):
                    fo = fout_pool.tile([128, 2 * D], BF16, tag="fout",
                                        name="fout")
                    for half in range(2):
                        t4 = pair * 2 + half
                        fsl = fo[:, half * D:(half + 1) * D]
                        if has_bias:
                            nc.vector.scalar_tensor_tensor(
                                fsl, fp_slices[t4], rec[:, t4:t4 + 1], bob,
                                op0=ALU.mult, op1=ALU.add)
                        elif half == 0:
                            nc.vector.tensor_scalar_mul(
                                fsl, fp_slices[t4], rec[:, t4:t4 + 1])
                        else:
                            nc.scalar.mul(fsl, fp_slices[t4],
                                          rec[:, t4:t4 + 1])
                    nc.sync.dma_start(out_g[p["qb"] * 2 + pair],
                                      fo.rearrange("p (j c) -> p j c", j=2))

    nc.compile()
    return nc


_NC = {}


def _get_nc(has_bias=False):
    if has_bias not in _NC:
        _NC[has_bias] = _build(has_bias=has_bias)
    return _NC[has_bias]


class _Runner:
    """Cached jitted SPMD executor (run_bass_kernel_spmd rebuilds its jax
    closure every call, forcing a retrace; this traces once)."""

    def __init__(self, nc):
        import jax
        from jax.sharding import Mesh, PartitionSpec
        from jax.experimental.shard_map import shard_map
        from concourse import bass2jax, mybir as mb

        bass2jax.install_neuronx_cc_hook()
        self.jax = jax
        if not any("axon" in str(getattr(d, "platform", "")).lower()
                   or str(d).startswith("NC_")
                   for d in jax.devices()):
            # jax was initialized on another platform (e.g. cpu for the
            # reference); reset so the axon NeuronCores are visible.
            import jax._src.xla_bridge as xb
            jax.config.update("jax_platforms", None)
            xb._clear_backends()
            if hasattr(xb.get_backend, "cache_clear"):
                xb.get_backend.cache_clear()
            if not any("axon" in str(getattr(d, "platform", "")).lower()
                       or str(d).startswith("NC_")
                       for d in jax.devices()):
                jax.config.update("jax_platforms", "axon")
                xb._clear_backends()
                if hasattr(xb.get_backend, "cache_clear"):
                    xb.get_backend.cache_clear()
        partition_name = (nc.partition_id_tensor.name
                          if nc.partition_id_tensor else None)
        in_names, out_names, out_avals = [], [], []
        for alloc in nc.m.functions[0].allocations:
            if not isinstance(alloc, mb.MemoryLocationSet):
                continue
            name = alloc.memorylocations[0].name
            if alloc.kind == "ExternalInput":
                if name != partition_name:
                    in_names.append(name)
            elif alloc.kind == "ExternalOutput":
                out_names.append(name)
                out_avals.append(jax.core.ShapedArray(
                    tuple(alloc.tensor_shape), mb.dt.np(alloc.dtype)))
        self.in_names, self.out_names, self.out_avals = \
            in_names, out_names, out_avals
        n_params, n_outs = len(in_names), len(out_names)
        bind_in_names = in_names + out_names + (
            [partition_name] if partition_name else [])

        def _body(*args):
            operands = list(args)
            if partition_name is not None:
                operands.append(bass2jax.partition_id_tensor())
            outs = bass2jax._bass_exec_p.bind(
                *operands,
                out_avals=tuple(out_avals),
                in_names=tuple(bind_in_names),
                out_names=tuple(out_names),
                lowering_input_output_aliases=(),
                sim_require_finite=True,
                sim_require_nnan=True,
                nc=nc,
            )
            return tuple(outs)

        devices = jax.devices()[:NCORES]
        mesh = Mesh(np.asarray(devices), ("core",))
        spec = (PartitionSpec("core"),) * (n_params + n_outs)
        self.fn = jax.jit(
            shard_map(_body, mesh=mesh, in_specs=spec,
                      out_specs=(PartitionSpec("core"),) * n_outs,
                      check_rep=False),
            donate_argnums=tuple(range(n_params, n_params + n_outs)),
            keep_unused=True,
        )

    def run(self, in_maps):
        concat_in = [
            np.concatenate([np.asarray(m[n]) for m in in_maps], axis=0)
            for n in self.in_names
        ]
        concat_zeros = [
            np.zeros((NCORES * a.shape[0], *a.shape[1:]), a.dtype)
            for a in self.out_avals
        ]
        outs = self.fn(*concat_in, *concat_zeros)
        return [
            {n: np.asarray(outs[i]).reshape(NCORES, *self.out_avals[i].shape)[c]
             for i, n in enumerate(self.out_names)}
            for c in range(NCORES)
        ]


_RUNNER = {}


def _get_runner(has_bias=False):
    if has_bias not in _RUNNER:
        _RUNNER[has_bias] = _Runner(_get_nc(has_bias))
    return _RUNNER[has_bias]


def _split8(a):
    """fp8 e4m3 hi/lo split: a ~= hi + lo elementwise."""
    hi = np.asarray(a, dtype=E4M3)
    lo = np.asarray(a.astype(np.float32) - hi.astype(np.float32), dtype=E4M3)
    return hi, lo


def _dstack(a):
    """[256, N] (d-major) -> [128, 2*N] chunk-stacked: row d_lo holds
    (chunk 0 cols, chunk 1 cols)."""
    n = a.shape[1]
    return np.ascontiguousarray(
        a.reshape(2, 128, n).transpose(1, 0, 2).reshape(128, 2 * n))


def make_in_maps(inputs):
    x = np.asarray(inputs["x"], dtype=np.float32)
    Wq = np.asarray(inputs["Wq"], dtype=np.float32)
    Wk = np.asarray(inputs["Wk"], dtype=np.float32)
    Wv = np.asarray(inputs["Wv"], dtype=np.float32)
    Wo = np.asarray(inputs["Wo"], dtype=np.float32)
    bq = np.asarray(inputs["bq"], dtype=np.float32)
    bv = np.asarray(inputs["bv"], dtype=np.float32)
    bo = np.asarray(inputs["bo"], dtype=np.float32)
    # bk drops out of softmax (per-query constant). bq only survives through
    # the per-key term x_k . (Wk bq), applied as an exp bias. bv folds into
    # the output bias (attention rows sum to 1).
    u8 = np.uint8
    M16 = (16.0 * (Wq @ Wk.T)).astype(np.float32)
    mh, ml = _split8(M16)
    wvh, wvl = _split8((16.0 * Wv).astype(np.float32))
    wpa = np.ascontiguousarray(np.concatenate(
        [_dstack(mh).view(u8), _dstack(ml).view(u8),
         _dstack(wvh).view(u8), _dstack(wvl).view(u8)], axis=1))
    wo_b = _dstack(np.asarray(Wo, dtype=ml_dtypes.bfloat16))
    bob = np.tile((bv @ Wo + bo).astype(np.float32)[None, :], (128, 1))
    ident = np.eye(128, dtype=ml_dtypes.bfloat16)
    wkbq = (Wk @ bq).astype(np.float32)
    in_maps = []
    for c in range(NCORES):
        b, h = divmod(c, 2)
        # Rotate the batch so this core's queries are rows 0..SQ-1; keys and
        # values see all rows either way (softmax is key-order invariant).
        xb = x[b] if h == 0 else np.ascontiguousarray(
            np.concatenate([x[b, SQ:], x[b, :SQ]]))
        xh, xl = _split8(xb)
        cvec = np.ascontiguousarray(
            (xb @ wkbq).astype(np.float32).reshape(32, 128).T)
        wp2b = np.ascontiguousarray(np.concatenate(
            [wo_b.view(u8), cvec.view(u8), bob.view(u8), ident.view(u8)],
            axis=1))
        assert wp2b.shape == (128, WP2_END)
        in_maps.append({
            "xpk": np.concatenate(
                [_dstack(np.ascontiguousarray(xh.T)),
                 _dstack(np.ascontiguousarray(xl.T))], axis=1),
            "wpa": wpa,
            "wp2b": wp2b,
        })
    return in_maps


def kernel(**inputs):
    bo_eff = (np.asarray(inputs["bv"], np.float32)
              @ np.asarray(inputs["Wo"], np.float32)
              + np.asarray(inputs["bo"], np.float32))
    has_bias = bool(np.any(bo_eff))
    try:
        runner = _get_runner(has_bias)
    except Exception:
        runner = None
    in_maps = make_in_maps(inputs)
    results = None
    if runner is not None:
        try:
            results = runner.run(in_maps)
        except Exception:
            results = None
    if results is None:
        results = run_bass_kernel_spmd(
            _get_nc(has_bias), in_maps,
            core_ids=list(range(NCORES))).results
    outp = np.empty((B, S, D), dtype=np.float32)
    for c in range(NCORES):
        b, h = divmod(c, 2)
        outp[b, h * SQ:(h + 1) * SQ] = \
            results[c]["out"].astype(np.float32)
    return outp
